# revision 23
# baseline (speedup 1.0000x reference)
"""PointPillars encoder on 8 Trainium2 NeuronCores.

Sharding: spatial over BEV rows (ny). Core k owns output rows [32k, 32k+32);
it receives the points falling in its halo-extended strip (rows 32k-4 .. 32k+36)
and computes its output strip independently (no collectives; the 4-row halo
covers the four 3x3 conv layers).

Device pipeline per core:
  1. MLP over raw points  (PE fp32r matmuls + ACT fused BN+ReLU)
  2. segment-max via "prefix planes": host orders points so that plane r holds
     the r-th point of every cell, cells sorted by descending count -> plane r
     is a prefix of plane r-1; the reduce is plain DVE tensor_max on SBUF.
  3. one indirect-DMA scatter places per-cell descriptors into the BEV grid.
  4. conv backbone: 3x3 convs as 9 shifted matmuls accumulating in PSUM,
     BN+ReLU fused on ACT.
"""

import numpy as np
from contextlib import ExitStack

import concourse.bass as bass
import concourse.bacc as bacc
import concourse.tile as tile
from concourse import mybir
from concourse._compat import get_trn_type
from concourse.bass import IndirectOffsetOnAxis
from concourse.bass_utils import run_bass_kernel_spmd
from concourse.masks import make_identity

F32 = mybir.dt.float32
F32R = mybir.dt.float32r
I32 = mybir.dt.int32
I16 = mybir.dt.int16
RELU = mybir.ActivationFunctionType.Relu

# ---- geometry (hardcoded from the problem spec) ----
NXG = 256            # global grid x
NYG = 256            # global grid y
ROWS = 40            # local strip rows (32 owned + 4 halo each side)
NCELL = ROWS * NXG   # 10240 local cells
HALO = 4
NCORES = 8

# ---- prefix planes ----
PRE = 512  # dummy zero-feature points; their MLP output == h_empty
PLANES = [10240, 9728, 6656, 3584, 2048, 1024] + [512] * 6  # r = 0..11
RMAX = len(PLANES)
NPTS = PRE + sum(PLANES)          # 38912
NTILES = NPTS // 512              # 76
PLANE_OFF = np.cumsum([PRE] + PLANES[:-1]).tolist()  # start offset of plane r

# tile t (of 512 points) -> (plane index r, column offset in plane) ; t=0 is PRE
_TILE_PLANE = {}
for _t in range(1, NTILES):
    _s = _t * 512
    for _r in range(RMAX):
        if PLANE_OFF[_r] <= _s < PLANE_OFF[_r] + PLANES[_r]:
            _TILE_PLANE[_t] = (_r, _s - PLANE_OFF[_r])
            break

# conv layers: (cin, cout), valid output row range [lo, hi) in local coords
CONV = [(64, 128, 1, 39), (128, 128, 2, 38), (128, 64, 3, 37), (64, 64, 4, 36)]

_CACHE = {}


def _build():
    nc = bacc.Bacc(get_trn_type() or "TRN2", target_bir_lowering=False, debug=False)

    feats = nc.dram_tensor("feats", [8, NPTS], F32, kind="ExternalInput")
    sidx = nc.dram_tensor("sidx", [128, 640], I16, kind="ExternalInput")
    w1 = nc.dram_tensor("w1", [8, 32], F32, kind="ExternalInput")
    w2 = nc.dram_tensor("w2", [32, 64], F32, kind="ExternalInput")
    bn1 = nc.dram_tensor("bn1", [32, 2], F32, kind="ExternalInput")
    bn2 = nc.dram_tensor("bn2", [64, 2], F32, kind="ExternalInput")
    cws = [
        nc.dram_tensor(f"cw{i}", [9, cin, cout], F32, kind="ExternalInput")
        for i, (cin, cout, _, _) in enumerate(CONV)
    ]
    cbns = [
        nc.dram_tensor(f"cbn{i}", [cout, 2], F32, kind="ExternalInput")
        for i, (_, cout, _, _) in enumerate(CONV)
    ]
    rowmask = nc.dram_tensor("rowmask", [128, 42], F32, kind="ExternalInput")
    out = nc.dram_tensor("out", [64, 32 * NXG], F32, kind="ExternalOutput")
    import os
    dbg = bool(os.environ.get("PP_DEBUG"))
    if dbg:
        dbg_accT = nc.dram_tensor("dbg_accT", [128, 5120], F32, kind="ExternalOutput")
        dbg_desc = nc.dram_tensor("dbg_desc", [128, 5120], F32, kind="ExternalOutput")
        dbg_x1 = nc.dram_tensor("dbg_x1", [64, 42 * 258], F32, kind="ExternalOutput")

    with tile.TileContext(nc) as tc:
        with tc.tile_pool(name="dram", bufs=1, space="DRAM") as dpool:
            desc = dpool.tile([NCELL + 128, 64], F32)

            # ---------------- phase A: MLP + segment max + scatter ----------
            with (
                tc.tile_pool(name="pa_const", bufs=1) as cpool,
                tc.tile_pool(name="pa_feats", bufs=2) as fpool,
                tc.tile_pool(name="pa_h", bufs=4) as hpool,
                tc.tile_pool(name="pa_acc", bufs=1) as apool,
                tc.tile_pool(name="pa_ps", bufs=2, space="PSUM") as pspool,
                tc.tile_pool(name="pa_pt", bufs=2, space="PSUM") as ptpool,
            ):
                w1_s = cpool.tile([8, 32], F32R)
                nc.sync.dma_start(w1_s[:], w1[:].bitcast(F32R))
                w2_s = cpool.tile([32, 64], F32R)
                nc.sync.dma_start(w2_s[:], w2[:].bitcast(F32R))
                bn1_s = cpool.tile([32, 2], F32)
                nc.sync.dma_start(bn1_s[:], bn1[:])
                bn2_s = cpool.tile([64, 2], F32)
                nc.sync.dma_start(bn2_s[:], bn2[:])
                ident = cpool.tile([128, 128], F32)
                make_identity(nc, ident[:])

                sidx_s = cpool.tile([128, 640], I16)
                nc.sync.dma_start(sidx_s[:], sidx[:])
                acc = apool.tile([64, NCELL], F32)
                accT = apool.tile([128, 80 * 64], F32)
                hcol = cpool.tile([64, 1], F32)

                # zero the grid early (no deps; overlaps the MLP)
                zt = cpool.tile([128, 1280], F32)
                nc.vector.memset(zt[:], 0.0)
                descz = desc[:NCELL, :].rearrange("(a b) c -> a (b c)", a=128)
                for i in range(4):
                    nc.sync.dma_start(descz[:, i * 1280:(i + 1) * 1280], zt[:])

                CH = 18 * 512  # feats chunk
                for c in range(4):
                    fch = fpool.tile([8, CH], F32R)
                    nc.sync.dma_start(
                        fch[:], feats[:, c * CH:(c + 1) * CH].bitcast(F32R)
                    )
                    for tt in range(18):
                        t = c * 18 + tt
                        ps1 = pspool.tile([32, 512], F32, tag="ps1")
                        nc.tensor.matmul(
                            ps1[:],
                            lhsT=w1_s[:],
                            rhs=fch[:, tt * 512:(tt + 1) * 512],
                            start=True, stop=True,
                        )
                        h1 = hpool.tile([32, 512], F32R, tag="h1")
                        nc.scalar.activation(
                            h1[:], ps1[:], RELU,
                            bias=bn1_s[:, 1:2], scale=bn1_s[:, 0:1],
                        )
                        ps2 = pspool.tile([64, 512], F32, tag="ps2")
                        nc.tensor.matmul(
                            ps2[:],
                            lhsT=w2_s[:],
                            rhs=h1[:],
                            start=True, stop=True,
                        )
                        h2 = hpool.tile([64, 512], F32, tag="h2")
                        nc.scalar.activation(
                            h2[:], ps2[:], RELU,
                            bias=bn2_s[:, 1:2], scale=bn2_s[:, 0:1],
                        )
                        if t == 0:
                            nc.vector.tensor_copy(hcol[:], h2[:, 0:1])
                        else:
                            r, pq = _TILE_PLANE[t]
                            if r == 0:
                                # acc = max(h2, h_empty)
                                nc.vector.tensor_scalar(
                                    out=acc[:, pq:pq + 512], in0=h2[:],
                                    scalar1=hcol[:, 0:1], scalar2=None,
                                    op0=mybir.AluOpType.max,
                                )
                            else:
                                nc.vector.tensor_max(
                                    acc[:, pq:pq + 512],
                                    acc[:, pq:pq + 512], h2[:],
                                )

                # transpose + scatter in descending order: tail columns are
                # finalized by early planes, so their chunks overlap the MLP
                for sc in reversed(range(NCELL // 1024)):
                    for g in range(8 * sc + 7, 8 * sc - 1, -1):
                        pt = ptpool.tile([128, 64], F32, tag="ptr")
                        nc.tensor.transpose(
                            pt[:], acc[:, g * 128:(g + 1) * 128], ident[:64, :64]
                        )
                        nc.vector.tensor_copy(accT[:, g * 64:(g + 1) * 64], pt[:])
                    nc.gpsimd.dma_scatter_add(
                        out_ap=desc[:],
                        in_ap=accT[:, sc * 8 * 64:(sc + 1) * 8 * 64].rearrange(
                            "p (g c) -> p g c", c=64
                        ),
                        idxs_ap=sidx_s[:, sc * 64:(sc + 1) * 64],
                        num_idxs=1024,
                        num_idxs_reg=1024,
                        elem_size=64,
                    )
                if dbg:
                    nc.sync.dma_start(dbg_accT[:], accT[:])

            # ---------------- phase B: conv backbone ------------------------
            with (
                tc.tile_pool(name="pb_const", bufs=1) as kpool,
                tc.tile_pool(name="pb_x", bufs=1) as xpool,
                tc.tile_pool(name="pb_rb", bufs=1) as rpool,
                tc.tile_pool(name="pb_out", bufs=4) as opool,
            ):
                ident2 = kpool.tile([128, 128], F32)
                make_identity(nc, ident2[:])
                rmask_s = kpool.tile([128, 42], F32)
                nc.sync.dma_start(rmask_s[:], rowmask[:])
                cw_s, cbn_s = [], []
                for i, (cin, cout, _, _) in enumerate(CONV):
                    w = kpool.tile([cin, 9 * cout], F32R, tag=f"cw{i}")
                    nc.sync.dma_start(
                        w[:].rearrange("i (s o) -> i s o", s=9),
                        cws[i][:].rearrange("s i o -> i s o").bitcast(F32R),
                    )
                    cw_s.append(w)
                    b = kpool.tile([cout, 2], F32, tag=f"cbn{i}")
                    nc.sync.dma_start(b[:], cbns[i][:])
                    cbn_s.append(b)

                # x buffers, padded [C, 42, 258]; x4 reuses x1's slot
                x1 = xpool.tile([64, 42, 258], F32R, tag="xa")
                x2 = xpool.tile([128, 42, 258], F32R, tag="xb")
                x3 = xpool.tile([128, 42, 258], F32R, tag="xc")
                xs = [x1, x2, x3, None]

                for xb in (x1, x2, x3):
                    nc.gpsimd.memset(xb[:, :, 0:1].bitcast(F32), 0.0)
                    nc.gpsimd.memset(xb[:, :, 257:258].bitcast(F32), 0.0)

                # readback grid (cell-strided) and transpose into x1 interior
                rb = rpool.tile([128, 80 * 64], F32)
                nc.sync.dma_start(
                    rb[:].rearrange("p (g c) -> p g c", c=64),
                    desc[:NCELL, :].rearrange("(g p) c -> p g c", p=128),
                )
                if dbg:
                    nc.sync.dma_start(
                        dbg_desc[:],
                        desc[:NCELL, :].rearrange("(a b) c -> a (b c)", a=128),
                    )
                with tc.tile_pool(name="pb_pt", bufs=4, space="PSUM") as cpt:
                    for g in range(80):
                        pt2 = cpt.tile([64, 128], F32, tag="ptc")
                        nc.tensor.transpose(
                            pt2[:], rb[:, g * 64:(g + 1) * 64], ident2[:]
                        )
                        row, colh = g // 2, (g % 2) * 128
                        nc.vector.tensor_copy(
                            x1[:, row + 1, 1 + colh:1 + colh + 128], pt2[:]
                        )

                if dbg:
                    nc.sync.dma_start(
                        dbg_x1[:], x1[:].bitcast(F32).rearrange("c a b -> c (a b)")
                    )
                with tc.tile_pool(name="pb_ps", bufs=8, space="PSUM") as cps:
                    self_conv(nc, tc, cps, xs, xpool, opool, cw_s, cbn_s, out, rmask_s)

    nc.compile()
    return nc


def self_conv(nc, tc, cps, xs, xpool, opool, cw_s, cbn_s, out, rmask_s):
    for li, (cin, cout, lo, hi) in enumerate(CONV):
        xin = xs[li]
        if li < 3:
            if li == 2:
                xout = xpool.tile([64, 42, 258], F32R, tag="xa")
                nc.gpsimd.memset(xout[:, :, 0:1].bitcast(F32), 0.0)
                nc.gpsimd.memset(xout[:, :, 257:258].bitcast(F32), 0.0)
                xs[3] = xout
            else:
                xout = xs[li + 1]
        if li > 0:
            # zero rows outside the global grid (edge cores only; the mask is
            # all-ones on interior cores). Reapplies SAME zero padding per
            # layer; slices cover exactly the written rows this layer reads.
            for a, b in {1: ((2, 5), (37, 40)),
                         2: ((3, 5), (37, 39)),
                         3: ((4, 5), (37, 38))}[li]:
                nc.vector.tensor_mul(
                    xin[:, a:b, :], xin[:, a:b, :],
                    rmask_s[:cin, a:b].to_broadcast([cin, b - a, 258]),
                )
        pairs = [(r, r + 1) for r in range(lo, hi, 2)]
        for gstart in range(0, len(pairs), 8):
            grp = pairs[gstart:gstart + 8]
            psts = [cps.tile([cout, 512], F32, tag="cps", name=f"cps{li}_{gstart}_{i}") for i, _ in enumerate(grp)]
            for s in range(9):
                dy, dx = s // 3, s % 3
                lhsT = cw_s[li][:, s * cout:(s + 1) * cout]
                for i, (r, _) in enumerate(grp):
                    nc.tensor.matmul(
                        psts[i][:],
                        lhsT=lhsT,
                        rhs=xin[:, r + dy:r + dy + 2, dx:dx + 256],
                        start=(s == 0), stop=(s == 8),
                    )
            for i, (r, _) in enumerate(grp):
                if li < 3:
                    nc.scalar.activation(
                        xout[:, r + 1:r + 3, 1:257], psts[i][:], RELU,
                        bias=cbn_s[li][:, 1:2], scale=cbn_s[li][:, 0:1],
                    )
                else:
                    ot = opool.tile([64, 512], F32, tag="ot")
                    nc.scalar.activation(
                        ot[:], psts[i][:], RELU,
                        bias=cbn_s[li][:, 1:2], scale=cbn_s[li][:, 0:1],
                    )
                    nc.sync.dma_start(
                        out[:, (r - 4) * 256:(r - 2) * 256], ot[:]
                    )


def _get_nc():
    if "nc" not in _CACHE:
        _CACHE["nc"] = _build()
    return _CACHE["nc"]


def host_prep(inputs):
    """Shard + order the inputs for the 8 cores. Pure numpy."""
    pts = np.asarray(inputs["points"], np.float32)
    n = pts.shape[0]
    x, y, z, it_ = pts[:, 0], pts[:, 1], pts[:, 2], pts[:, 3]
    inb = (x >= -51.2) & (x < 51.2) & (y >= -51.2) & (y < 51.2)
    xi = np.clip(np.floor((x + 51.2) / 0.4).astype(np.int64), 0, NXG - 1)
    yi = np.clip(np.floor((y + 51.2) / 0.4).astype(np.int64), 0, NYG - 1)
    flat = np.where(inb, yi * NXG + xi, NXG * NYG)
    order = np.argsort(flat, kind="stable")
    sf = flat[order]
    idxs = np.arange(n)
    is_start = np.concatenate([[True], sf[1:] != sf[:-1]])
    seg_start = np.maximum.accumulate(np.where(is_start, idxs, 0))
    slot = np.empty(n, np.int64)
    slot[order] = idxs - seg_start
    valid = inb & (slot < 32)

    cx = xi.astype(np.float32) * 0.4 + np.float32(-51.2 + 0.2)
    cy = yi.astype(np.float32) * 0.4 + np.float32(-51.2 + 0.2)
    feats7 = np.stack([x, y, z, it_, x - cx, y - cy, z], 0).astype(np.float32)

    w1a = np.zeros((8, 32), np.float32)
    w1a[:7] = np.asarray(inputs["w1"], np.float32)
    shared = {
        "w1": w1a,
        "w2": np.asarray(inputs["w2"], np.float32),
        "bn1": np.stack([inputs["s1"], inputs["t1"]], 1).astype(np.float32),
        "bn2": np.stack([inputs["s2"], inputs["t2"]], 1).astype(np.float32),
    }
    for i, nmw, nms, nmt in (
        (0, "cw1a", "cs1a", "ct1a"),
        (1, "cw1b", "cs1b", "ct1b"),
        (2, "cw2a", "cs2a", "ct2a"),
        (3, "cw2b", "cs2b", "ct2b"),
    ):
        cw = np.asarray(inputs[nmw], np.float32)  # [O, I, 3, 3]
        shared[f"cw{i}"] = np.ascontiguousarray(
            cw.transpose(2, 3, 1, 0).reshape(9, cw.shape[1], cw.shape[0])
        )
        shared[f"cbn{i}"] = np.stack(
            [inputs[nms], inputs[nmt]], 1
        ).astype(np.float32)

    in_maps = []
    for k in range(NCORES):
        lo, hi = 32 * k - HALO, 32 * k + 32 + HALO
        sel = valid & (yi >= lo) & (yi < hi)
        lcell = ((yi[sel] - lo) * NXG + xi[sel]).astype(np.int64)
        slot_s = slot[sel]
        f_s = feats7[:, sel]
        cnt = np.bincount(lcell, minlength=NCELL)
        assert cnt.max() <= RMAX, f"cell occupancy {cnt.max()} > {RMAX}"
        nonempty = np.nonzero(cnt)[0]
        ordcells = nonempty[np.lexsort((nonempty, -cnt[nonempty]))]
        ordinal = np.full(NCELL, -1, np.int64)
        ordinal[ordcells] = np.arange(len(ordcells))
        for r in range(RMAX):
            nr = int((cnt > r).sum())
            assert nr <= PLANES[r], f"plane {r}: {nr} > {PLANES[r]}"
        feats8 = np.zeros((8, NPTS), np.float32)
        pos = np.asarray(PLANE_OFF)[slot_s] + ordinal[lcell]
        feats8[:7, pos] = f_s
        # scatter destination per sorted ordinal: its grid cell; padding
        # ordinals accumulate onto the dummy row NCELL
        sj = np.full(NCELL, NCELL, np.int64)
        sj[: len(ordcells)] = ordcells
        s16 = sj.reshape(640, 16).T.astype(np.int16)        # [16, 640] wrap
        sidx = np.ascontiguousarray(np.tile(s16, (8, 1)))   # replicate to 128
        grow = lo + np.arange(42) - 1  # global row of padded-buffer row pr
        rowmask = ((grow >= 0) & (grow < NYG)).astype(np.float32)
        rowmask = np.broadcast_to(rowmask, (128, 42)).copy()
        in_maps.append(
            {"feats": feats8, "sidx": sidx, "rowmask": rowmask, **shared}
        )
    return in_maps


def kernel(**inputs):
    import os
    in_maps = host_prep(inputs)
    nc = _get_nc()
    trace = bool(os.environ.get("PP_TRACE"))
    res = run_bass_kernel_spmd(
        nc, in_maps, core_ids=list(range(NCORES)), trace=trace
    )
    _CACHE["last_result"] = res
    strips = [r["out"].reshape(64, 32, NXG) for r in res.results]
    full = np.concatenate(strips, axis=1)
    return np.ascontiguousarray(full[None]).astype(np.float32)


# revision 25
# speedup vs baseline: 1.1174x; 1.1174x over previous
"""PointPillars encoder on 8 Trainium2 NeuronCores.

Sharding: spatial over BEV rows (ny). Core k owns output rows [32k, 32k+32);
it receives the points falling in its halo-extended strip (rows 32k-4 .. 32k+36)
and computes its output strip independently (no collectives; the 4-row halo
covers the four 3x3 conv layers).

Device pipeline per core:
  1. MLP over raw points  (PE fp32r matmuls + ACT fused BN+ReLU)
  2. segment-max via "prefix planes": host orders points so that plane r holds
     the r-th point of every cell, cells sorted by descending count -> plane r
     is a prefix of plane r-1; the reduce is plain DVE tensor_max on SBUF.
  3. one indirect-DMA scatter places per-cell descriptors into the BEV grid.
  4. conv backbone: 3x3 convs as 9 shifted matmuls accumulating in PSUM,
     BN+ReLU fused on ACT.
"""

import numpy as np
from contextlib import ExitStack

import concourse.bass as bass
import concourse.bacc as bacc
import concourse.tile as tile
from concourse import mybir
from concourse._compat import get_trn_type
from concourse.bass import IndirectOffsetOnAxis
from concourse.bass_utils import run_bass_kernel_spmd
from concourse.masks import make_identity

F32 = mybir.dt.float32
F32R = mybir.dt.float32r
I32 = mybir.dt.int32
I16 = mybir.dt.int16
RELU = mybir.ActivationFunctionType.Relu

# ---- geometry (hardcoded from the problem spec) ----
NXG = 256            # global grid x
NYG = 256            # global grid y
ROWS = 40            # local strip rows (32 owned + 4 halo each side)
NCELL = ROWS * NXG   # 10240 local cells
HALO = 4
NCORES = 8

# ---- prefix planes ----
PRE = 512  # dummy zero-feature points; their MLP output == h_empty
PLANES = [10240, 9728, 6656, 3584, 2048, 1024] + [512] * 6  # r = 0..11
RMAX = len(PLANES)
NPTS = PRE + sum(PLANES)          # 38912
NTILES = NPTS // 512              # 76
PLANE_OFF = np.cumsum([PRE] + PLANES[:-1]).tolist()  # start offset of plane r

# tile t (of 512 points) -> (plane index r, column offset in plane) ; t=0 is PRE
_TILE_PLANE = {}
for _t in range(1, NTILES):
    _s = _t * 512
    for _r in range(RMAX):
        if PLANE_OFF[_r] <= _s < PLANE_OFF[_r] + PLANES[_r]:
            _TILE_PLANE[_t] = (_r, _s - PLANE_OFF[_r])
            break

# conv layers: (cin, cout), valid output row range [lo, hi) in local coords
CONV = [(64, 128, 1, 39), (128, 128, 2, 38), (128, 64, 3, 37), (64, 64, 4, 36)]

_CACHE = {}


def _build():
    nc = bacc.Bacc(get_trn_type() or "TRN2", target_bir_lowering=False, debug=False)

    feats = nc.dram_tensor("feats", [8, NPTS], F32, kind="ExternalInput")
    sidx = nc.dram_tensor("sidx", [128, 640], I16, kind="ExternalInput")
    w1 = nc.dram_tensor("w1", [8, 32], F32, kind="ExternalInput")
    w2 = nc.dram_tensor("w2", [32, 64], F32, kind="ExternalInput")
    bn1 = nc.dram_tensor("bn1", [32, 2], F32, kind="ExternalInput")
    bn2 = nc.dram_tensor("bn2", [64, 2], F32, kind="ExternalInput")
    cws = [
        nc.dram_tensor(f"cw{i}", [9, cin, cout], F32, kind="ExternalInput")
        for i, (cin, cout, _, _) in enumerate(CONV)
    ]
    cbns = [
        nc.dram_tensor(f"cbn{i}", [cout, 2], F32, kind="ExternalInput")
        for i, (_, cout, _, _) in enumerate(CONV)
    ]
    rowmask = nc.dram_tensor("rowmask", [128, 42], F32, kind="ExternalInput")
    out = nc.dram_tensor("out", [64, 32 * NXG], F32, kind="ExternalOutput")
    import os
    dbg = bool(os.environ.get("PP_DEBUG"))
    if dbg:
        dbg_accT = nc.dram_tensor("dbg_accT", [128, 5120], F32, kind="ExternalOutput")
        dbg_desc = nc.dram_tensor("dbg_desc", [128, 5120], F32, kind="ExternalOutput")
        dbg_x1 = nc.dram_tensor("dbg_x1", [64, 42 * 258], F32, kind="ExternalOutput")

    with tile.TileContext(nc) as tc:
        with tc.tile_pool(name="dram", bufs=1, space="DRAM") as dpool:
            desc = dpool.tile([NCELL + 128, 64], F32)
            descB = dpool.tile([NCELL + 128, 64], F32)

            # ---------------- phase A: MLP + segment max + scatter ----------
            with (
                tc.tile_pool(name="pa_const", bufs=1) as cpool,
                tc.tile_pool(name="pa_feats", bufs=2) as fpool,
                tc.tile_pool(name="pa_h", bufs=4) as hpool,
                tc.tile_pool(name="pa_acc", bufs=1) as apool,
                tc.tile_pool(name="pa_ps", bufs=2, space="PSUM") as pspool,
                tc.tile_pool(name="pa_pt", bufs=2, space="PSUM") as ptpool,
            ):
                w1_s = cpool.tile([8, 32], F32R)
                nc.sync.dma_start(w1_s[:], w1[:].bitcast(F32R))
                w2_s = cpool.tile([32, 64], F32R)
                nc.sync.dma_start(w2_s[:], w2[:].bitcast(F32R))
                bn1_s = cpool.tile([32, 2], F32)
                nc.sync.dma_start(bn1_s[:], bn1[:])
                bn2_s = cpool.tile([64, 2], F32)
                nc.sync.dma_start(bn2_s[:], bn2[:])
                ident = cpool.tile([128, 128], F32)
                make_identity(nc, ident[:])

                sidx_s = cpool.tile([128, 640], I16)
                nc.sync.dma_start(sidx_s[:], sidx[:])
                acc = apool.tile([64, NCELL], F32)
                accT = apool.tile([128, 80 * 64], F32)
                hcol = cpool.tile([64, 1], F32)

                # zero the grid early (no deps; overlaps the MLP)
                zt = cpool.tile([128, 1280], F32)
                nc.vector.memset(zt[:], 0.0)
                descz = desc[:NCELL, :].rearrange("(a b) c -> a (b c)", a=128)
                descBz = descB[:NCELL, :].rearrange("(a b) c -> a (b c)", a=128)
                for i in range(4):
                    nc.sync.dma_start(descz[:, i * 1280:(i + 1) * 1280], zt[:])
                    nc.scalar.dma_start(descBz[:, i * 1280:(i + 1) * 1280], zt[:])

                CH = 18 * 512  # feats chunk
                for c in range(4):
                    fch = fpool.tile([8, CH], F32R)
                    nc.sync.dma_start(
                        fch[:], feats[:, c * CH:(c + 1) * CH].bitcast(F32R)
                    )
                    for tt in range(18):
                        t = c * 18 + tt
                        ps1 = pspool.tile([32, 512], F32, tag="ps1")
                        nc.tensor.matmul(
                            ps1[:],
                            lhsT=w1_s[:],
                            rhs=fch[:, tt * 512:(tt + 1) * 512],
                            start=True, stop=True,
                        )
                        h1 = hpool.tile([32, 512], F32R, tag="h1")
                        nc.scalar.activation(
                            h1[:], ps1[:], RELU,
                            bias=bn1_s[:, 1:2], scale=bn1_s[:, 0:1],
                        )
                        ps2 = pspool.tile([64, 512], F32, tag="ps2")
                        nc.tensor.matmul(
                            ps2[:],
                            lhsT=w2_s[:],
                            rhs=h1[:],
                            start=True, stop=True,
                        )
                        h2 = hpool.tile([64, 512], F32, tag="h2")
                        nc.scalar.activation(
                            h2[:], ps2[:], RELU,
                            bias=bn2_s[:, 1:2], scale=bn2_s[:, 0:1],
                        )
                        if t == 0:
                            nc.vector.tensor_copy(hcol[:], h2[:, 0:1])
                        else:
                            r, pq = _TILE_PLANE[t]
                            if r == 0:
                                # acc = max(h2, h_empty)
                                nc.vector.tensor_scalar(
                                    out=acc[:, pq:pq + 512], in0=h2[:],
                                    scalar1=hcol[:, 0:1], scalar2=None,
                                    op0=mybir.AluOpType.max,
                                )
                            else:
                                nc.vector.tensor_max(
                                    acc[:, pq:pq + 512],
                                    acc[:, pq:pq + 512], h2[:],
                                )

                # transpose + scatter in descending order: tail columns are
                # finalized by early planes, so their chunks overlap the MLP
                for sc in reversed(range(NCELL // 1024)):
                    for g in range(8 * sc + 7, 8 * sc - 1, -1):
                        pt = ptpool.tile([128, 64], F32, tag="ptr")
                        nc.tensor.transpose(
                            pt[:], acc[:, g * 128:(g + 1) * 128], ident[:64, :64]
                        )
                        nc.vector.tensor_copy(accT[:, g * 64:(g + 1) * 64], pt[:])
                    nc.gpsimd.dma_scatter_add(
                        out_ap=desc[:] if sc % 2 == 0 else descB[:],
                        in_ap=accT[:, sc * 8 * 64:(sc + 1) * 8 * 64].rearrange(
                            "p (g c) -> p g c", c=64
                        ),
                        idxs_ap=sidx_s[:, sc * 64:(sc + 1) * 64],
                        num_idxs=1024,
                        num_idxs_reg=1024,
                        elem_size=64,
                    )
                if dbg:
                    nc.sync.dma_start(dbg_accT[:], accT[:])

            # ---------------- phase B: conv backbone ------------------------
            with (
                tc.tile_pool(name="pb_const", bufs=1) as kpool,
                tc.tile_pool(name="pb_x", bufs=1) as xpool,
                tc.tile_pool(name="pb_rb", bufs=1) as rpool,
                tc.tile_pool(name="pb_out", bufs=4) as opool,
            ):
                ident2 = kpool.tile([128, 128], F32)
                make_identity(nc, ident2[:])
                rmask_s = kpool.tile([128, 42], F32)
                nc.sync.dma_start(rmask_s[:], rowmask[:])
                cw_s, cbn_s = [], []
                for i, (cin, cout, _, _) in enumerate(CONV):
                    w = kpool.tile([cin, 9 * cout], F32R, tag=f"cw{i}")
                    nc.sync.dma_start(
                        w[:].rearrange("i (s o) -> i s o", s=9),
                        cws[i][:].rearrange("s i o -> i s o").bitcast(F32R),
                    )
                    cw_s.append(w)
                    b = kpool.tile([cout, 2], F32, tag=f"cbn{i}")
                    nc.sync.dma_start(b[:], cbns[i][:])
                    cbn_s.append(b)

                # x buffers, padded [C, 42, 258]; x4 reuses x1's slot
                x1 = xpool.tile([64, 42, 258], F32R, tag="xa")
                x2 = xpool.tile([128, 42, 258], F32R, tag="xb")
                x3 = xpool.tile([128, 42, 258], F32R, tag="xc")
                xs = [x1, x2, x3, None]

                for xb in (x1, x2, x3):
                    nc.gpsimd.memset(xb[:, :, 0:1].bitcast(F32), 0.0)
                    nc.gpsimd.memset(xb[:, :, 257:258].bitcast(F32), 0.0)

                # readback grid (cell-strided) and transpose into x1 interior
                rbA = rpool.tile([128, 80 * 64], F32)
                nc.sync.dma_start(
                    rbA[:].rearrange("p (g c) -> p g c", c=64),
                    desc[:NCELL, :].rearrange("(g p) c -> p g c", p=128),
                )
                rbB = rpool.tile([128, 80 * 64], F32)
                nc.scalar.dma_start(
                    rbB[:].rearrange("p (g c) -> p g c", c=64),
                    descB[:NCELL, :].rearrange("(g p) c -> p g c", p=128),
                )
                rb = rbA
                nc.vector.tensor_add(rbA[:], rbA[:], rbB[:])
                if dbg:
                    nc.sync.dma_start(
                        dbg_desc[:],
                        desc[:NCELL, :].rearrange("(a b) c -> a (b c)", a=128),
                    )
                with tc.tile_pool(name="pb_pt", bufs=4, space="PSUM") as cpt:
                    for g in range(80):
                        pt2 = cpt.tile([64, 128], F32, tag="ptc")
                        nc.tensor.transpose(
                            pt2[:], rb[:, g * 64:(g + 1) * 64], ident2[:]
                        )
                        row, colh = g // 2, (g % 2) * 128
                        nc.vector.tensor_copy(
                            x1[:, row + 1, 1 + colh:1 + colh + 128], pt2[:]
                        )

                if dbg:
                    nc.sync.dma_start(
                        dbg_x1[:], x1[:].bitcast(F32).rearrange("c a b -> c (a b)")
                    )
                with tc.tile_pool(name="pb_ps", bufs=8, space="PSUM") as cps:
                    self_conv(nc, tc, cps, xs, xpool, opool, cw_s, cbn_s, out, rmask_s)

    nc.compile()
    return nc


def self_conv(nc, tc, cps, xs, xpool, opool, cw_s, cbn_s, out, rmask_s):
    for li, (cin, cout, lo, hi) in enumerate(CONV):
        xin = xs[li]
        if li < 3:
            if li == 2:
                xout = xpool.tile([64, 42, 258], F32R, tag="xa")
                nc.gpsimd.memset(xout[:, :, 0:1].bitcast(F32), 0.0)
                nc.gpsimd.memset(xout[:, :, 257:258].bitcast(F32), 0.0)
                xs[3] = xout
            else:
                xout = xs[li + 1]
        if li > 0:
            # zero rows outside the global grid (edge cores only; the mask is
            # all-ones on interior cores). Reapplies SAME zero padding per
            # layer; slices cover exactly the written rows this layer reads.
            for a, b in {1: ((2, 5), (37, 40)),
                         2: ((3, 5), (37, 39)),
                         3: ((4, 5), (37, 38))}[li]:
                nc.vector.tensor_mul(
                    xin[:, a:b, :], xin[:, a:b, :],
                    rmask_s[:cin, a:b].to_broadcast([cin, b - a, 258]),
                )
        pairs = [(r, r + 1) for r in range(lo, hi, 2)]
        for gstart in range(0, len(pairs), 8):
            grp = pairs[gstart:gstart + 8]
            psts = [cps.tile([cout, 512], F32, tag="cps", name=f"cps{li}_{gstart}_{i}") for i, _ in enumerate(grp)]
            for s in range(9):
                dy, dx = s // 3, s % 3
                lhsT = cw_s[li][:, s * cout:(s + 1) * cout]
                for i, (r, _) in enumerate(grp):
                    nc.tensor.matmul(
                        psts[i][:],
                        lhsT=lhsT,
                        rhs=xin[:, r + dy:r + dy + 2, dx:dx + 256],
                        start=(s == 0), stop=(s == 8),
                    )
            for i, (r, _) in enumerate(grp):
                if li < 3:
                    nc.scalar.activation(
                        xout[:, r + 1:r + 3, 1:257], psts[i][:], RELU,
                        bias=cbn_s[li][:, 1:2], scale=cbn_s[li][:, 0:1],
                    )
                else:
                    ot = opool.tile([64, 512], F32, tag="ot")
                    nc.scalar.activation(
                        ot[:], psts[i][:], RELU,
                        bias=cbn_s[li][:, 1:2], scale=cbn_s[li][:, 0:1],
                    )
                    nc.sync.dma_start(
                        out[:, (r - 4) * 256:(r - 2) * 256], ot[:]
                    )


def _get_nc():
    if "nc" not in _CACHE:
        _CACHE["nc"] = _build()
    return _CACHE["nc"]


def host_prep(inputs):
    """Shard + order the inputs for the 8 cores. Pure numpy."""
    pts = np.asarray(inputs["points"], np.float32)
    n = pts.shape[0]
    x, y, z, it_ = pts[:, 0], pts[:, 1], pts[:, 2], pts[:, 3]
    inb = (x >= -51.2) & (x < 51.2) & (y >= -51.2) & (y < 51.2)
    xi = np.clip(np.floor((x + 51.2) / 0.4).astype(np.int64), 0, NXG - 1)
    yi = np.clip(np.floor((y + 51.2) / 0.4).astype(np.int64), 0, NYG - 1)
    flat = np.where(inb, yi * NXG + xi, NXG * NYG)
    order = np.argsort(flat, kind="stable")
    sf = flat[order]
    idxs = np.arange(n)
    is_start = np.concatenate([[True], sf[1:] != sf[:-1]])
    seg_start = np.maximum.accumulate(np.where(is_start, idxs, 0))
    slot = np.empty(n, np.int64)
    slot[order] = idxs - seg_start
    valid = inb & (slot < 32)

    cx = xi.astype(np.float32) * 0.4 + np.float32(-51.2 + 0.2)
    cy = yi.astype(np.float32) * 0.4 + np.float32(-51.2 + 0.2)
    feats7 = np.stack([x, y, z, it_, x - cx, y - cy, z], 0).astype(np.float32)

    w1a = np.zeros((8, 32), np.float32)
    w1a[:7] = np.asarray(inputs["w1"], np.float32)
    shared = {
        "w1": w1a,
        "w2": np.asarray(inputs["w2"], np.float32),
        "bn1": np.stack([inputs["s1"], inputs["t1"]], 1).astype(np.float32),
        "bn2": np.stack([inputs["s2"], inputs["t2"]], 1).astype(np.float32),
    }
    for i, nmw, nms, nmt in (
        (0, "cw1a", "cs1a", "ct1a"),
        (1, "cw1b", "cs1b", "ct1b"),
        (2, "cw2a", "cs2a", "ct2a"),
        (3, "cw2b", "cs2b", "ct2b"),
    ):
        cw = np.asarray(inputs[nmw], np.float32)  # [O, I, 3, 3]
        shared[f"cw{i}"] = np.ascontiguousarray(
            cw.transpose(2, 3, 1, 0).reshape(9, cw.shape[1], cw.shape[0])
        )
        shared[f"cbn{i}"] = np.stack(
            [inputs[nms], inputs[nmt]], 1
        ).astype(np.float32)

    in_maps = []
    for k in range(NCORES):
        lo, hi = 32 * k - HALO, 32 * k + 32 + HALO
        sel = valid & (yi >= lo) & (yi < hi)
        lcell = ((yi[sel] - lo) * NXG + xi[sel]).astype(np.int64)
        slot_s = slot[sel]
        f_s = feats7[:, sel]
        cnt = np.bincount(lcell, minlength=NCELL)
        assert cnt.max() <= RMAX, f"cell occupancy {cnt.max()} > {RMAX}"
        nonempty = np.nonzero(cnt)[0]
        ordcells = nonempty[np.lexsort((nonempty, -cnt[nonempty]))]
        ordinal = np.full(NCELL, -1, np.int64)
        ordinal[ordcells] = np.arange(len(ordcells))
        for r in range(RMAX):
            nr = int((cnt > r).sum())
            assert nr <= PLANES[r], f"plane {r}: {nr} > {PLANES[r]}"
        feats8 = np.zeros((8, NPTS), np.float32)
        pos = np.asarray(PLANE_OFF)[slot_s] + ordinal[lcell]
        feats8[:7, pos] = f_s
        # scatter destination per sorted ordinal: its grid cell; padding
        # ordinals accumulate onto the dummy row NCELL
        sj = np.full(NCELL, NCELL, np.int64)
        sj[: len(ordcells)] = ordcells
        s16 = sj.reshape(640, 16).T.astype(np.int16)        # [16, 640] wrap
        sidx = np.ascontiguousarray(np.tile(s16, (8, 1)))   # replicate to 128
        grow = lo + np.arange(42) - 1  # global row of padded-buffer row pr
        rowmask = ((grow >= 0) & (grow < NYG)).astype(np.float32)
        rowmask = np.broadcast_to(rowmask, (128, 42)).copy()
        in_maps.append(
            {"feats": feats8, "sidx": sidx, "rowmask": rowmask, **shared}
        )
    return in_maps


def kernel(**inputs):
    import os
    in_maps = host_prep(inputs)
    nc = _get_nc()
    trace = bool(os.environ.get("PP_TRACE"))
    res = run_bass_kernel_spmd(
        nc, in_maps, core_ids=list(range(NCORES)), trace=trace
    )
    _CACHE["last_result"] = res
    strips = [r["out"].reshape(64, 32, NXG) for r in res.results]
    full = np.concatenate(strips, axis=1)
    return np.ascontiguousarray(full[None]).astype(np.float32)
